# revision 17
# baseline (speedup 1.0000x reference)
"""Trainium2 Bass kernel for nn_KernelGraphCalcLayer (GNN message passing).

Computation (per batch b):
    h = relu(node_feats @ weight + bias)            # (N, OUT_DIM)
    h = h.reshape(N, K, DK)
    out[n, k, d] = sum_m adj[k, n, m] * h[m, k, d]  # per-kernel dense aggregation

Sharding: batch dim (64) split across 8 NeuronCores, 8 batches per core.
No cross-device communication.

Per-core dataflow:
  - adj (16MB, bulk of HBM traffic) loads via HWDGE at full rate, declared
    float32r (same bits as fp32): PE is_transpose runs at 1.5 cyc/row instead
    of 2.0, and transposition is pure data movement so it stays bit-exact.
    The PSUM->SBUF copy casts to bf16 (VectorE, 1/3 on ScalarE for balance).
  - x (4MB) + W load via SWDGE cast-DMA to bf16 (SWDGE otherwise idle);
    xT comes from hardware DMA-transpose (xbar, 2-byte) on the Scalar HWDGE
    queue -- no PE or VectorE involvement.
  - Linear: psum_h[128,512] = ones.T @ bias (K=1 seed) + sum_i xT_i.T @ W_i,
    ScalarE relu -> h bf16.
  - Aggregation: per node-chunk a full PSUM bank [128,512] accumulates all
    8 kernel slots (2 matmuls each); a single VectorE copy drains it.
"""

import numpy as np

import concourse.bass as bass
import concourse.mybir as mybir
from concourse import bacc
import concourse.tile as tile
from concourse.bass_utils import run_bass_kernel_spmd
from concourse.masks import make_identity

B, N, IN_DIM, OUT_DIM, K = 64, 256, 512, 512, 8
DK = OUT_DIM // K
N_CORES = 8
BPC = B // N_CORES  # batches per core

FP32 = mybir.dt.float32
FP32R = mybir.dt.float32r
CDT = mybir.dt.bfloat16  # compute dtype for matmul operands
P = 128  # SBUF partitions

_compiled = {}


def _build(cdt=CDT):
    nc = bacc.Bacc("TRN2", target_bir_lowering=False, debug=False)
    x_ap = nc.dram_tensor("node_feats", [BPC, N, IN_DIM], FP32, kind="ExternalInput").ap()
    adj_ap = nc.dram_tensor("adj", [BPC, K, N, N], FP32R, kind="ExternalInput").ap()
    w_ap = nc.dram_tensor("weight", [IN_DIM, OUT_DIM], FP32, kind="ExternalInput").ap()
    b_ap = nc.dram_tensor("bias", [OUT_DIM], FP32, kind="ExternalInput").ap()
    out_ap = nc.dram_tensor("out", [BPC, N, OUT_DIM], FP32, kind="ExternalOutput").ap()

    NC2 = N // P       # 2 node chunks of 128
    IC4 = IN_DIM // P  # 4 input-feature chunks

    with tile.TileContext(nc) as tc:
        with (
            tc.tile_pool(name="singles", bufs=1) as singles,
            tc.tile_pool(name="p_x", bufs=3) as p_x,
            tc.tile_pool(name="p_xt", bufs=8) as p_xt,
            tc.tile_pool(name="p_h", bufs=4) as p_h,
            tc.tile_pool(name="p_adj", bufs=12) as p_adj,
            tc.tile_pool(name="p_adjt", bufs=8) as p_adjt,
            tc.tile_pool(name="p_out", bufs=4) as p_out,
            tc.tile_pool(name="ps_ta", bufs=4, space=bass.MemorySpace.PSUM) as ps_ta,
            tc.tile_pool(name="ps_h", bufs=2, space=bass.MemorySpace.PSUM) as ps_h,
            tc.tile_pool(name="ps_o", bufs=2, space=bass.MemorySpace.PSUM) as ps_o,
        ):
            # --- constants ---
            id_src = singles.tile([P, P], FP32)
            make_identity(nc, id_src[:])
            id_f = singles.tile([P, P], FP32R)    # identity for fp32r transposes
            nc.vector.tensor_copy(id_f[:], id_src[:])
            id_c = singles.tile([P, P], cdt)      # identity for bf16 transposes
            make_identity(nc, id_c[:])
            ones_row = singles.tile([1, P], cdt)
            nc.gpsimd.memset(ones_row[:], 1.0)
            bias_c = singles.tile([1, OUT_DIM], cdt)
            nc.gpsimd.dma_start(out=bias_c[:], in_=b_ap[None, :])
            w_sb = [singles.tile([P, OUT_DIM], cdt, name=f"w{ic}")
                    for ic in range(IC4)]

            # DRAM views packing the leading 256 rows into [128, 2, cols]
            x_v = x_ap.rearrange("b (c p) i -> b p c i", p=P)      # [BPC,128,2,512]
            adj_v = adj_ap.rearrange("b k (c p) m -> b k p c m", p=P)

            cast_rr = 0  # round-robin DVE/ACT for adjT casts

            for b in range(BPC):
                # --- prefetch adj[b, k] fp32r via HWDGE, packed [128, 2*256] ---
                a_sbs = []
                for k in range(K):
                    a_sb = p_adj.tile([P, NC2 * N], FP32R, tag="adj",
                                      name=f"a{b}_{k}")
                    eng = nc.sync if k % 2 == 0 else nc.scalar
                    eng.dma_start(out=a_sb[:], in_=adj_v[b, k])
                    a_sbs.append(a_sb)

                # --- x: SWDGE cast load bf16, packed [128, 2*512] ---
                x_sb = p_x.tile([P, NC2 * IN_DIM], cdt, tag="x", name=f"x{b}")
                nc.gpsimd.dma_start(out=x_sb[:], in_=x_v[b])

                # --- transpose x -> xT packed [128(i), 2ic x 256(n)] (bf16) ---
                # two ic chunks share one PSUM bank; one copy drains both
                xTp = []
                for icp in range(IC4 // 2):
                    t = p_xt.tile([P, 2 * N], cdt, tag="xT", name=f"xT{b}_{icp}")
                    pt = ps_ta.tile([P, 2 * N], cdt, tag="pstf",
                                    name=f"ptx{b}_{icp}")
                    for ici in range(2):
                        ic = icp * 2 + ici
                        for nch in range(NC2):
                            nc.tensor.transpose(
                                pt[:, ici * N + nch * P:
                                   ici * N + (nch + 1) * P],
                                x_sb[:, nch * IN_DIM + ic * P:
                                     nch * IN_DIM + (ic + 1) * P],
                                id_c[:])
                    nc.vector.tensor_copy(t[:], pt[:])
                    xTp.append(t)

                if b == 0:
                    # W loads issue after batch 0's x is in flight: the
                    # linear is the first consumer, ~15us into the kernel
                    for ic in range(IC4):
                        nc.gpsimd.dma_start(
                            out=w_sb[ic][:], in_=w_ap[ic * P:(ic + 1) * P, :])

                def xT_sl(ic, nch):
                    return xTp[ic // 2][:, (ic % 2) * N + nch * P:
                                        (ic % 2) * N + (nch + 1) * P]

                # --- linear + bias + relu -> h bf16 [128(n), 512(o)] x2 ---
                h_sb = []
                for nch in range(NC2):
                    ph = ps_h.tile([P, OUT_DIM], FP32, tag="psh", name=f"ph{b}_{nch}")
                    nc.tensor.matmul(ph[:], ones_row[:], bias_c[:],
                                     start=True, stop=False)
                    for ic in range(IC4):
                        nc.tensor.matmul(
                            ph[:], xT_sl(ic, nch), w_sb[ic][:],
                            start=False, stop=(ic == IC4 - 1))
                    ht = p_h.tile([P, OUT_DIM], cdt, tag="h", name=f"h{b}_{nch}")
                    nc.scalar.activation(ht[:], ph[:],
                                         mybir.ActivationFunctionType.Relu)
                    h_sb.append(ht)

                # --- per-kernel aggregation ---
                # full-bank accumulators: all 8 kernel slots land in one bank
                po = [ps_o.tile([P, OUT_DIM], FP32, tag="pso", name=f"po{b}_{i}")
                      for i in range(NC2)]
                for k in range(K):
                    a_sb = a_sbs[k]
                    # transpose -> adjT packed [128(m), 2mch x 256(n)] bf16;
                    # all 4 transposes share one PSUM bank, one cast drains it
                    aT = p_adjt.tile([P, 2 * N], cdt, tag="adjT",
                                     name=f"aT{b}_{k}")
                    pt = ps_ta.tile([P, 2 * N], FP32R, tag="pstf",
                                    name=f"pta{b}_{k}")
                    for mch in range(NC2):
                        for nch in range(NC2):
                            nc.tensor.transpose(
                                pt[:, mch * N + nch * P:
                                   mch * N + (nch + 1) * P],
                                a_sb[:, nch * N + mch * P:
                                     nch * N + (mch + 1) * P],
                                id_f[:])
                    if cast_rr % 4 == 3:
                        nc.scalar.copy(aT[:], pt[:])
                    else:
                        nc.vector.tensor_copy(aT[:], pt[:])
                    cast_rr += 1
                    # po[n, k*DK:+DK] = sum_m adjT[m,n].T @ h[m, k*DK:+DK]
                    for nch in range(NC2):
                        for mch in range(NC2):
                            nc.tensor.matmul(
                                po[nch][:, k * DK:(k + 1) * DK],
                                aT[:, mch * N + nch * P:
                                   mch * N + (nch + 1) * P],
                                h_sb[mch][:, k * DK:(k + 1) * DK],
                                start=(mch == 0), stop=(mch == NC2 - 1))

                # --- drain accumulators + store ---
                for nch in range(NC2):
                    ot = p_out.tile([P, OUT_DIM], FP32, tag="o", name=f"o{b}_{nch}")
                    nc.vector.tensor_copy(ot[:], po[nch][:])
                    nc.scalar.dma_start(
                        out=out_ap[b, nch * P:(nch + 1) * P, :], in_=ot[:])

    nc.compile()
    return nc


def _get_nc():
    if "nc" not in _compiled:
        _compiled["nc"] = _build()
    return _compiled["nc"]


def _run(inputs, trace=False, trace_cores=None):
    nc = _get_nc()
    node_feats = np.ascontiguousarray(inputs["node_feats"], dtype=np.float32)
    adj = np.ascontiguousarray(inputs["adj"], dtype=np.float32)
    weight = np.ascontiguousarray(inputs["weight"], dtype=np.float32)
    bias = np.ascontiguousarray(inputs["bias"], dtype=np.float32)
    in_maps = []
    for c in range(N_CORES):
        sl = slice(c * BPC, (c + 1) * BPC)
        in_maps.append({
            "node_feats": node_feats[sl],
            "adj": adj[sl],
            "weight": weight,
            "bias": bias,
        })
    res = run_bass_kernel_spmd(
        nc, in_maps, core_ids=list(range(N_CORES)),
        trace=trace, trace_cores=trace_cores)
    out = np.concatenate([res.results[c]["out"] for c in range(N_CORES)], axis=0)
    return out.reshape(B, N, OUT_DIM), res


def kernel(**inputs) -> np.ndarray:
    return _run(inputs, trace=False)[0]


# revision 18
# speedup vs baseline: 1.1149x; 1.1149x over previous
"""Trainium2 Bass kernel for nn_KernelGraphCalcLayer (GNN message passing).

Computation (per batch b):
    h = relu(node_feats @ weight + bias)            # (N, OUT_DIM)
    h = h.reshape(N, K, DK)
    out[n, k, d] = sum_m adj[k, n, m] * h[m, k, d]  # per-kernel dense aggregation

Sharding: batch dim (64) split across 8 NeuronCores, 8 batches per core.
No cross-device communication.

Per-core dataflow:
  - adj (16MB, bulk of HBM traffic) loads via HWDGE at full rate, declared
    float32r (same bits as fp32): PE is_transpose runs at 1.5 cyc/row instead
    of 2.0, and transposition is pure data movement so it stays bit-exact.
    The PSUM->SBUF copy casts to bf16 (VectorE, 1/3 on ScalarE for balance).
  - x (4MB) + W load via SWDGE cast-DMA to bf16 (SWDGE otherwise idle);
    xT comes from hardware DMA-transpose (xbar, 2-byte) on the Scalar HWDGE
    queue -- no PE or VectorE involvement.
  - Linear: psum_h[128,512] = ones.T @ bias (K=1 seed) + sum_i xT_i.T @ W_i,
    ScalarE relu -> h bf16.
  - Aggregation: per node-chunk a full PSUM bank [128,512] accumulates all
    8 kernel slots (2 matmuls each); a single VectorE copy drains it.
"""

import numpy as np

import concourse.bass as bass
import concourse.mybir as mybir
from concourse import bacc
import concourse.tile as tile
from concourse.bass_utils import run_bass_kernel_spmd
from concourse.masks import make_identity

B, N, IN_DIM, OUT_DIM, K = 64, 256, 512, 512, 8
DK = OUT_DIM // K
N_CORES = 8
BPC = B // N_CORES  # batches per core

FP32 = mybir.dt.float32
FP32R = mybir.dt.float32r
CDT = mybir.dt.bfloat16  # compute dtype for matmul operands
P = 128  # SBUF partitions

_compiled = {}


def _build(cdt=CDT):
    nc = bacc.Bacc("TRN2", target_bir_lowering=False, debug=False)
    x_ap = nc.dram_tensor("node_feats", [BPC, N, IN_DIM], FP32R, kind="ExternalInput").ap()
    adj_ap = nc.dram_tensor("adj", [BPC, K, N, N], FP32R, kind="ExternalInput").ap()
    w_ap = nc.dram_tensor("weight", [IN_DIM, OUT_DIM], FP32, kind="ExternalInput").ap()
    b_ap = nc.dram_tensor("bias", [OUT_DIM], FP32, kind="ExternalInput").ap()
    out_ap = nc.dram_tensor("out", [BPC, N, OUT_DIM], FP32, kind="ExternalOutput").ap()

    NC2 = N // P       # 2 node chunks of 128
    IC4 = IN_DIM // P  # 4 input-feature chunks

    with tile.TileContext(nc) as tc:
        with (
            tc.tile_pool(name="singles", bufs=1) as singles,
            tc.tile_pool(name="p_x", bufs=3) as p_x,
            tc.tile_pool(name="p_xt", bufs=8) as p_xt,
            tc.tile_pool(name="p_h", bufs=4) as p_h,
            tc.tile_pool(name="p_adj", bufs=12) as p_adj,
            tc.tile_pool(name="p_adjt", bufs=8) as p_adjt,
            tc.tile_pool(name="p_out", bufs=4) as p_out,
            tc.tile_pool(name="ps_ta", bufs=4, space=bass.MemorySpace.PSUM) as ps_ta,
            tc.tile_pool(name="ps_h", bufs=2, space=bass.MemorySpace.PSUM) as ps_h,
            tc.tile_pool(name="ps_o", bufs=2, space=bass.MemorySpace.PSUM) as ps_o,
        ):
            # --- constants ---
            id_src = singles.tile([P, P], FP32)
            make_identity(nc, id_src[:])
            id_f = singles.tile([P, P], FP32R)    # identity for fp32r transposes
            nc.vector.tensor_copy(id_f[:], id_src[:])
            id_c = singles.tile([P, P], cdt)      # identity for bf16 transposes
            make_identity(nc, id_c[:])
            ones_row = singles.tile([1, P], cdt)
            nc.gpsimd.memset(ones_row[:], 1.0)
            bias_c = singles.tile([1, OUT_DIM], cdt)
            nc.gpsimd.dma_start(out=bias_c[:], in_=b_ap[None, :])
            w_sb = [singles.tile([P, OUT_DIM], cdt, name=f"w{ic}")
                    for ic in range(IC4)]

            # DRAM views packing the leading 256 rows into [128, 2, cols]
            x_v = x_ap.rearrange("b (c p) i -> b p c i", p=P)      # [BPC,128,2,512]
            adj_v = adj_ap.rearrange("b k (c p) m -> b k p c m", p=P)

            cast_rr = 0  # round-robin DVE/ACT for adjT casts

            for b in range(BPC):
                # --- prefetch adj[b, k] fp32r via HWDGE, packed [128, 2*256] ---
                a_sbs = []
                for k in range(K):
                    a_sb = p_adj.tile([P, NC2 * N], FP32R, tag="adj",
                                      name=f"a{b}_{k}")
                    nc.sync.dma_start(out=a_sb[:], in_=adj_v[b, k])
                    a_sbs.append(a_sb)

                # --- x: HWDGE fp32r load, packed [128, 2*512] ---
                x_sb = p_x.tile([P, NC2 * IN_DIM], FP32R, tag="x", name=f"x{b}")
                nc.scalar.dma_start(out=x_sb[:], in_=x_v[b])

                # --- transpose x -> xT packed [128(i), 2ic x 256(n)] (bf16) ---
                # two ic chunks share one PSUM bank; one copy drains both
                xTp = []
                for icp in range(IC4 // 2):
                    t = p_xt.tile([P, 2 * N], cdt, tag="xT", name=f"xT{b}_{icp}")
                    pt = ps_ta.tile([P, 2 * N], FP32R, tag="pstf",
                                    name=f"ptx{b}_{icp}")
                    for ici in range(2):
                        ic = icp * 2 + ici
                        for nch in range(NC2):
                            nc.tensor.transpose(
                                pt[:, ici * N + nch * P:
                                   ici * N + (nch + 1) * P],
                                x_sb[:, nch * IN_DIM + ic * P:
                                     nch * IN_DIM + (ic + 1) * P],
                                id_f[:])
                    nc.vector.tensor_copy(t[:], pt[:])
                    xTp.append(t)

                if b == 0:
                    # W loads issue after batch 0's x is in flight: the
                    # linear is the first consumer, ~15us into the kernel
                    for ic in range(IC4):
                        nc.gpsimd.dma_start(
                            out=w_sb[ic][:], in_=w_ap[ic * P:(ic + 1) * P, :])

                def xT_sl(ic, nch):
                    return xTp[ic // 2][:, (ic % 2) * N + nch * P:
                                        (ic % 2) * N + (nch + 1) * P]

                # --- linear + bias + relu -> h bf16 [128(n), 512(o)] x2 ---
                h_sb = []
                for nch in range(NC2):
                    ph = ps_h.tile([P, OUT_DIM], FP32, tag="psh", name=f"ph{b}_{nch}")
                    nc.tensor.matmul(ph[:], ones_row[:], bias_c[:],
                                     start=True, stop=False)
                    for ic in range(IC4):
                        nc.tensor.matmul(
                            ph[:], xT_sl(ic, nch), w_sb[ic][:],
                            start=False, stop=(ic == IC4 - 1))
                    ht = p_h.tile([P, OUT_DIM], cdt, tag="h", name=f"h{b}_{nch}")
                    nc.scalar.activation(ht[:], ph[:],
                                         mybir.ActivationFunctionType.Relu)
                    h_sb.append(ht)

                # --- per-kernel aggregation ---
                # full-bank accumulators: all 8 kernel slots land in one bank
                po = [ps_o.tile([P, OUT_DIM], FP32, tag="pso", name=f"po{b}_{i}")
                      for i in range(NC2)]
                for k in range(K):
                    a_sb = a_sbs[k]
                    # transpose -> adjT packed [128(m), 2mch x 256(n)] bf16;
                    # all 4 transposes share one PSUM bank, one cast drains it
                    aT = p_adjt.tile([P, 2 * N], cdt, tag="adjT",
                                     name=f"aT{b}_{k}")
                    pt = ps_ta.tile([P, 2 * N], FP32R, tag="pstf",
                                    name=f"pta{b}_{k}")
                    for mch in range(NC2):
                        for nch in range(NC2):
                            nc.tensor.transpose(
                                pt[:, mch * N + nch * P:
                                   mch * N + (nch + 1) * P],
                                a_sb[:, nch * N + mch * P:
                                     nch * N + (mch + 1) * P],
                                id_f[:])
                    if cast_rr % 4 == 3:
                        nc.scalar.copy(aT[:], pt[:])
                    else:
                        nc.vector.tensor_copy(aT[:], pt[:])
                    cast_rr += 1
                    # po[n, k*DK:+DK] = sum_m adjT[m,n].T @ h[m, k*DK:+DK]
                    for nch in range(NC2):
                        for mch in range(NC2):
                            nc.tensor.matmul(
                                po[nch][:, k * DK:(k + 1) * DK],
                                aT[:, mch * N + nch * P:
                                   mch * N + (nch + 1) * P],
                                h_sb[mch][:, k * DK:(k + 1) * DK],
                                start=(mch == 0), stop=(mch == NC2 - 1))

                # --- drain accumulators + store ---
                for nch in range(NC2):
                    ot = p_out.tile([P, OUT_DIM], FP32, tag="o", name=f"o{b}_{nch}")
                    nc.vector.tensor_copy(ot[:], po[nch][:])
                    nc.scalar.dma_start(
                        out=out_ap[b, nch * P:(nch + 1) * P, :], in_=ot[:])

    nc.compile()
    return nc


def _get_nc():
    if "nc" not in _compiled:
        _compiled["nc"] = _build()
    return _compiled["nc"]


def _run(inputs, trace=False, trace_cores=None):
    nc = _get_nc()
    node_feats = np.ascontiguousarray(inputs["node_feats"], dtype=np.float32)
    adj = np.ascontiguousarray(inputs["adj"], dtype=np.float32)
    weight = np.ascontiguousarray(inputs["weight"], dtype=np.float32)
    bias = np.ascontiguousarray(inputs["bias"], dtype=np.float32)
    in_maps = []
    for c in range(N_CORES):
        sl = slice(c * BPC, (c + 1) * BPC)
        in_maps.append({
            "node_feats": node_feats[sl],
            "adj": adj[sl],
            "weight": weight,
            "bias": bias,
        })
    res = run_bass_kernel_spmd(
        nc, in_maps, core_ids=list(range(N_CORES)),
        trace=trace, trace_cores=trace_cores)
    out = np.concatenate([res.results[c]["out"] for c in range(N_CORES)], axis=0)
    return out.reshape(B, N, OUT_DIM), res


def kernel(**inputs) -> np.ndarray:
    return _run(inputs, trace=False)[0]


# revision 19
# speedup vs baseline: 1.1658x; 1.0456x over previous
"""Trainium2 Bass kernel for nn_KernelGraphCalcLayer (GNN message passing).

Computation (per batch b):
    h = relu(node_feats @ weight + bias)            # (N, OUT_DIM)
    h = h.reshape(N, K, DK)
    out[n, k, d] = sum_m adj[k, n, m] * h[m, k, d]  # per-kernel dense aggregation

Sharding: batch dim (64) split across 8 NeuronCores, 8 batches per core.
No cross-device communication.

Per-core dataflow:
  - adj (16MB, bulk of HBM traffic) loads via HWDGE at full rate, declared
    float32r (same bits as fp32): PE is_transpose runs at 1.5 cyc/row instead
    of 2.0, and transposition is pure data movement so it stays bit-exact.
    The PSUM->SBUF copy casts to bf16 (VectorE, 1/3 on ScalarE for balance).
  - x (4MB) + W load via SWDGE cast-DMA to bf16 (SWDGE otherwise idle);
    xT comes from hardware DMA-transpose (xbar, 2-byte) on the Scalar HWDGE
    queue -- no PE or VectorE involvement.
  - Linear: psum_h[128,512] = ones.T @ bias (K=1 seed) + sum_i xT_i.T @ W_i,
    ScalarE relu -> h bf16.
  - Aggregation: per node-chunk a full PSUM bank [128,512] accumulates all
    8 kernel slots (2 matmuls each); a single VectorE copy drains it.
"""

import numpy as np

import concourse.bass as bass
import concourse.mybir as mybir
from concourse import bacc
import concourse.tile as tile
from concourse.bass_utils import run_bass_kernel_spmd
from concourse.masks import make_identity

B, N, IN_DIM, OUT_DIM, K = 64, 256, 512, 512, 8
DK = OUT_DIM // K
N_CORES = 8
BPC = B // N_CORES  # batches per core

FP32 = mybir.dt.float32
FP32R = mybir.dt.float32r
CDT = mybir.dt.bfloat16  # compute dtype for matmul operands
P = 128  # SBUF partitions

_compiled = {}


def _build(cdt=CDT):
    nc = bacc.Bacc("TRN2", target_bir_lowering=False, debug=False)
    x_ap = nc.dram_tensor("node_feats", [BPC, N, IN_DIM], FP32, kind="ExternalInput").ap()
    adj_ap = nc.dram_tensor("adj", [BPC, K, N, N], FP32R, kind="ExternalInput").ap()
    w_ap = nc.dram_tensor("weight", [IN_DIM, OUT_DIM], FP32, kind="ExternalInput").ap()
    b_ap = nc.dram_tensor("bias", [OUT_DIM], FP32, kind="ExternalInput").ap()
    out_ap = nc.dram_tensor("out", [BPC, N, OUT_DIM], FP32, kind="ExternalOutput").ap()

    NC2 = N // P       # 2 node chunks of 128
    IC4 = IN_DIM // P  # 4 input-feature chunks

    with tile.TileContext(nc) as tc:
        with (
            tc.tile_pool(name="singles", bufs=1) as singles,
            tc.tile_pool(name="p_x", bufs=3) as p_x,
            tc.tile_pool(name="p_xt", bufs=8) as p_xt,
            tc.tile_pool(name="p_h", bufs=4) as p_h,
            tc.tile_pool(name="p_adj", bufs=12) as p_adj,
            tc.tile_pool(name="p_adjt", bufs=8) as p_adjt,
            tc.tile_pool(name="p_out", bufs=4) as p_out,
            tc.tile_pool(name="ps_ta", bufs=6, space=bass.MemorySpace.PSUM) as ps_ta,
            tc.tile_pool(name="ps_o", bufs=2, space=bass.MemorySpace.PSUM) as ps_o,
        ):
            # --- constants ---
            id_src = singles.tile([P, P], FP32)
            make_identity(nc, id_src[:])
            id_f = singles.tile([P, P], FP32R)    # identity for fp32r transposes
            nc.vector.tensor_copy(id_f[:], id_src[:])
            id_c = singles.tile([P, P], cdt)      # identity for bf16 transposes
            make_identity(nc, id_c[:])
            ones_row = singles.tile([1, P], cdt)
            nc.gpsimd.memset(ones_row[:], 1.0)
            bias_c = singles.tile([1, OUT_DIM], cdt)
            nc.gpsimd.dma_start(out=bias_c[:], in_=b_ap[None, :])
            w_sb = [singles.tile([P, OUT_DIM], cdt, name=f"w{ic}")
                    for ic in range(IC4)]

            # DRAM views packing the leading 256 rows into [128, 2, cols]
            x_v = x_ap.rearrange("b (c p) i -> b p c i", p=P)      # [BPC,128,2,512]
            adj_v = adj_ap.rearrange("b k (c p) m -> b k p c m", p=P)

            cast_rr = 0  # round-robin DVE/ACT for adjT casts

            for b in range(BPC):
                # --- prefetch adj[b, k] fp32r via HWDGE, packed [128, 2*256] ---
                a_sbs = []
                for k in range(K):
                    a_sb = p_adj.tile([P, NC2 * N], FP32R, tag="adj",
                                      name=f"a{b}_{k}")
                    nc.sync.dma_start(out=a_sb[:], in_=adj_v[b, k])
                    a_sbs.append(a_sb)

                # --- x: SWDGE cast load bf16, packed [128, 2*512] ---
                x_sb = p_x.tile([P, NC2 * IN_DIM], cdt, tag="x", name=f"x{b}")
                nc.gpsimd.dma_start(out=x_sb[:], in_=x_v[b])

                # --- transpose x -> xT packed [128(i), 2ic x 256(n)] (bf16) ---
                # two ic chunks share one PSUM bank; one copy drains both
                xTp = []
                for icp in range(IC4 // 2):
                    t = p_xt.tile([P, 2 * N], cdt, tag="xT", name=f"xT{b}_{icp}")
                    pt = ps_ta.tile([P, 2 * N], cdt, tag="pstf",
                                    name=f"ptx{b}_{icp}")
                    for ici in range(2):
                        ic = icp * 2 + ici
                        for nch in range(NC2):
                            nc.tensor.transpose(
                                pt[:, ici * N + nch * P:
                                   ici * N + (nch + 1) * P],
                                x_sb[:, nch * IN_DIM + ic * P:
                                     nch * IN_DIM + (ic + 1) * P],
                                id_c[:])
                    nc.vector.tensor_copy(t[:], pt[:])
                    xTp.append(t)

                if b == 0:
                    # W loads issue after batch 0's x is in flight: the
                    # linear is the first consumer, ~15us into the kernel
                    for ic in range(IC4):
                        nc.gpsimd.dma_start(
                            out=w_sb[ic][:], in_=w_ap[ic * P:(ic + 1) * P, :])

                def xT_sl(ic, nch):
                    return xTp[ic // 2][:, (ic % 2) * N + nch * P:
                                        (ic % 2) * N + (nch + 1) * P]

                # --- linear + bias + relu -> h bf16 [128(n), 512(o)] x2 ---
                h_sb = []
                for nch in range(NC2):
                    ph = ps_ta.tile([P, OUT_DIM], FP32, tag="pstf", name=f"ph{b}_{nch}")
                    nc.tensor.matmul(ph[:], ones_row[:], bias_c[:],
                                     start=True, stop=False)
                    for ic in range(IC4):
                        nc.tensor.matmul(
                            ph[:], xT_sl(ic, nch), w_sb[ic][:],
                            start=False, stop=(ic == IC4 - 1))
                    ht = p_h.tile([P, OUT_DIM], cdt, tag="h", name=f"h{b}_{nch}")
                    nc.scalar.activation(ht[:], ph[:],
                                         mybir.ActivationFunctionType.Relu)
                    h_sb.append(ht)

                # --- per-kernel aggregation ---
                # full-bank accumulators: all 8 kernel slots land in one bank
                po = [ps_o.tile([P, OUT_DIM], FP32, tag="pso", name=f"po{b}_{i}")
                      for i in range(NC2)]
                for k in range(K):
                    a_sb = a_sbs[k]
                    # transpose -> adjT packed [128(m), 2mch x 256(n)] bf16;
                    # all 4 transposes share one PSUM bank, one cast drains it
                    aT = p_adjt.tile([P, 2 * N], cdt, tag="adjT",
                                     name=f"aT{b}_{k}")
                    pt = ps_ta.tile([P, 2 * N], FP32R, tag="pstf",
                                    name=f"pta{b}_{k}")
                    for mch in range(NC2):
                        for nch in range(NC2):
                            nc.tensor.transpose(
                                pt[:, mch * N + nch * P:
                                   mch * N + (nch + 1) * P],
                                a_sb[:, nch * N + mch * P:
                                     nch * N + (mch + 1) * P],
                                id_f[:])
                    if cast_rr % 4 == 3:
                        nc.scalar.copy(aT[:], pt[:])
                    else:
                        nc.vector.tensor_copy(aT[:], pt[:])
                    cast_rr += 1
                    # po[n, k*DK:+DK] = sum_m adjT[m,n].T @ h[m, k*DK:+DK]
                    for nch in range(NC2):
                        for mch in range(NC2):
                            nc.tensor.matmul(
                                po[nch][:, k * DK:(k + 1) * DK],
                                aT[:, mch * N + nch * P:
                                   mch * N + (nch + 1) * P],
                                h_sb[mch][:, k * DK:(k + 1) * DK],
                                start=(mch == 0), stop=(mch == NC2 - 1))

                # --- drain accumulators + store ---
                for nch in range(NC2):
                    ot = p_out.tile([P, OUT_DIM], FP32, tag="o", name=f"o{b}_{nch}")
                    nc.vector.tensor_copy(ot[:], po[nch][:])
                    nc.scalar.dma_start(
                        out=out_ap[b, nch * P:(nch + 1) * P, :], in_=ot[:])

    nc.compile()
    return nc


def _get_nc():
    if "nc" not in _compiled:
        _compiled["nc"] = _build()
    return _compiled["nc"]


def _run(inputs, trace=False, trace_cores=None):
    nc = _get_nc()
    node_feats = np.ascontiguousarray(inputs["node_feats"], dtype=np.float32)
    adj = np.ascontiguousarray(inputs["adj"], dtype=np.float32)
    weight = np.ascontiguousarray(inputs["weight"], dtype=np.float32)
    bias = np.ascontiguousarray(inputs["bias"], dtype=np.float32)
    in_maps = []
    for c in range(N_CORES):
        sl = slice(c * BPC, (c + 1) * BPC)
        in_maps.append({
            "node_feats": node_feats[sl],
            "adj": adj[sl],
            "weight": weight,
            "bias": bias,
        })
    res = run_bass_kernel_spmd(
        nc, in_maps, core_ids=list(range(N_CORES)),
        trace=trace, trace_cores=trace_cores)
    out = np.concatenate([res.results[c]["out"] for c in range(N_CORES)], axis=0)
    return out.reshape(B, N, OUT_DIM), res


def kernel(**inputs) -> np.ndarray:
    return _run(inputs, trace=False)[0]


# revision 20
# speedup vs baseline: 1.2134x; 1.0408x over previous
"""Trainium2 Bass kernel for nn_KernelGraphCalcLayer (GNN message passing).

Computation (per batch b):
    h = relu(node_feats @ weight + bias)            # (N, OUT_DIM)
    h = h.reshape(N, K, DK)
    out[n, k, d] = sum_m adj[k, n, m] * h[m, k, d]  # per-kernel dense aggregation

Sharding: batch dim (64) split across 8 NeuronCores, 8 batches per core.
No cross-device communication.

Per-core dataflow:
  - adj (16MB, bulk of HBM traffic) loads via HWDGE at full rate, declared
    float32r (same bits as fp32): PE is_transpose runs at 1.5 cyc/row instead
    of 2.0, and transposition is pure data movement so it stays bit-exact.
    The PSUM->SBUF copy casts to bf16 (VectorE, 1/3 on ScalarE for balance).
  - x (4MB) + W load via SWDGE cast-DMA to bf16 (SWDGE otherwise idle);
    xT comes from hardware DMA-transpose (xbar, 2-byte) on the Scalar HWDGE
    queue -- no PE or VectorE involvement.
  - Linear: psum_h[128,512] = ones.T @ bias (K=1 seed) + sum_i xT_i.T @ W_i,
    ScalarE relu -> h bf16.
  - Aggregation: per node-chunk a full PSUM bank [128,512] accumulates all
    8 kernel slots (2 matmuls each); a single VectorE copy drains it.
"""

import numpy as np

import concourse.bass as bass
import concourse.mybir as mybir
from concourse import bacc
import concourse.tile as tile
from concourse.bass_utils import run_bass_kernel_spmd
from concourse.masks import make_identity

B, N, IN_DIM, OUT_DIM, K = 64, 256, 512, 512, 8
DK = OUT_DIM // K
N_CORES = 8
BPC = B // N_CORES  # batches per core

FP32 = mybir.dt.float32
FP32R = mybir.dt.float32r
CDT = mybir.dt.bfloat16  # compute dtype for matmul operands
P = 128  # SBUF partitions

_compiled = {}


def _build(cdt=CDT):
    nc = bacc.Bacc("TRN2", target_bir_lowering=False, debug=False)
    x_ap = nc.dram_tensor("node_feats", [BPC, N, IN_DIM], FP32, kind="ExternalInput").ap()
    adj_ap = nc.dram_tensor("adj", [BPC, K, N, N], FP32R, kind="ExternalInput").ap()
    w_ap = nc.dram_tensor("weight", [IN_DIM, OUT_DIM], FP32, kind="ExternalInput").ap()
    b_ap = nc.dram_tensor("bias", [OUT_DIM], FP32, kind="ExternalInput").ap()
    out_ap = nc.dram_tensor("out", [BPC, N, OUT_DIM], FP32, kind="ExternalOutput").ap()

    NC2 = N // P       # 2 node chunks of 128
    IC4 = IN_DIM // P  # 4 input-feature chunks

    with tile.TileContext(nc) as tc:
        with (
            tc.tile_pool(name="singles", bufs=1) as singles,
            tc.tile_pool(name="p_x", bufs=3) as p_x,
            tc.tile_pool(name="p_xt", bufs=8) as p_xt,
            tc.tile_pool(name="p_h", bufs=4) as p_h,
            tc.tile_pool(name="p_adj", bufs=12) as p_adj,
            tc.tile_pool(name="p_adjt", bufs=8) as p_adjt,
            tc.tile_pool(name="p_out", bufs=4) as p_out,
            tc.tile_pool(name="ps_ta", bufs=4, space=bass.MemorySpace.PSUM) as ps_ta,
            tc.tile_pool(name="ps_h", bufs=2, space=bass.MemorySpace.PSUM) as ps_h,
            tc.tile_pool(name="ps_o", bufs=2, space=bass.MemorySpace.PSUM) as ps_o,
        ):
            # --- constants ---
            id_src = singles.tile([P, P], FP32)
            make_identity(nc, id_src[:])
            id_f = singles.tile([P, P], FP32R)    # identity for fp32r transposes
            nc.vector.tensor_copy(id_f[:], id_src[:])
            id_c = singles.tile([P, P], cdt)      # identity for bf16 transposes
            make_identity(nc, id_c[:])
            ones_row = singles.tile([1, P], cdt)
            nc.gpsimd.memset(ones_row[:], 1.0)
            bias_c = singles.tile([1, OUT_DIM], cdt)
            nc.gpsimd.dma_start(out=bias_c[:], in_=b_ap[None, :])
            w_sb = [singles.tile([P, OUT_DIM], cdt, name=f"w{ic}")
                    for ic in range(IC4)]

            # DRAM views packing the leading 256 rows into [128, 2, cols]
            x_v = x_ap.rearrange("b (c p) i -> b p c i", p=P)      # [BPC,128,2,512]
            adj_v = adj_ap.rearrange("b k (c p) m -> b k p c m", p=P)

            cast_rr = 0  # round-robin DVE/ACT for adjT casts

            for b in range(BPC):
                # --- prefetch adj[b, k] fp32r via HWDGE, packed [128, 2*256] ---
                a_sbs = []
                for k in range(K):
                    a_sb = p_adj.tile([P, NC2 * N], FP32R, tag="adj",
                                      name=f"a{b}_{k}")
                    nc.sync.dma_start(out=a_sb[:], in_=adj_v[b, k])
                    a_sbs.append(a_sb)

                # --- x: SWDGE cast load bf16, packed [128, 2*512] ---
                x_sb = p_x.tile([P, NC2 * IN_DIM], cdt, tag="x", name=f"x{b}")
                nc.gpsimd.dma_start(out=x_sb[:], in_=x_v[b])

                # --- transpose x -> xT packed [128(i), 2ic x 256(n)] (bf16) ---
                # two ic chunks share one PSUM bank; one copy drains both
                xTp = []
                for icp in range(IC4 // 2):
                    t = p_xt.tile([P, 2 * N], cdt, tag="xT", name=f"xT{b}_{icp}")
                    pt = ps_ta.tile([P, 2 * N], cdt, tag="pstf",
                                    name=f"ptx{b}_{icp}")
                    for ici in range(2):
                        ic = icp * 2 + ici
                        for nch in range(NC2):
                            nc.tensor.transpose(
                                pt[:, ici * N + nch * P:
                                   ici * N + (nch + 1) * P],
                                x_sb[:, nch * IN_DIM + ic * P:
                                     nch * IN_DIM + (ic + 1) * P],
                                id_c[:])
                    nc.vector.tensor_copy(t[:], pt[:])
                    xTp.append(t)

                if b == 0:
                    # W loads issue after batch 0's x is in flight: the
                    # linear is the first consumer, ~15us into the kernel
                    for ic in range(IC4):
                        nc.gpsimd.dma_start(
                            out=w_sb[ic][:], in_=w_ap[ic * P:(ic + 1) * P, :])

                def xT_sl(ic, nch):
                    return xTp[ic // 2][:, (ic % 2) * N + nch * P:
                                        (ic % 2) * N + (nch + 1) * P]

                # --- linear + bias + relu -> h bf16 [128(n), 512(o)] x2 ---
                h_sb = []
                for nch in range(NC2):
                    ph = ps_h.tile([P, OUT_DIM], FP32, tag="psh", name=f"ph{b}_{nch}")
                    nc.tensor.matmul(ph[:], ones_row[:], bias_c[:],
                                     start=True, stop=False)
                    for ic in range(IC4):
                        nc.tensor.matmul(
                            ph[:], xT_sl(ic, nch), w_sb[ic][:],
                            start=False, stop=(ic == IC4 - 1))
                    ht = p_h.tile([P, OUT_DIM], cdt, tag="h", name=f"h{b}_{nch}")
                    nc.scalar.activation(ht[:], ph[:],
                                         mybir.ActivationFunctionType.Relu)
                    h_sb.append(ht)

                # --- per-kernel aggregation ---
                # full-bank accumulators: all 8 kernel slots land in one bank
                po = [ps_o.tile([P, OUT_DIM], FP32, tag="pso", name=f"po{b}_{i}")
                      for i in range(NC2)]
                for k in range(K):
                    a_sb = a_sbs[k]
                    # transpose -> adjT packed [128(m), 2mch x 256(n)] bf16;
                    # all 4 transposes share one PSUM bank, one cast drains it
                    aT = p_adjt.tile([P, 2 * N], cdt, tag="adjT",
                                     name=f"aT{b}_{k}")
                    pt = ps_ta.tile([P, 2 * N], FP32R, tag="pstf",
                                    name=f"pta{b}_{k}")
                    for mch in range(NC2):
                        for nch in range(NC2):
                            nc.tensor.transpose(
                                pt[:, mch * N + nch * P:
                                   mch * N + (nch + 1) * P],
                                a_sb[:, nch * N + mch * P:
                                     nch * N + (mch + 1) * P],
                                id_f[:])
                    if cast_rr % 4 == 3:
                        nc.scalar.copy(aT[:], pt[:])
                    else:
                        nc.vector.tensor_copy(aT[:], pt[:])
                    cast_rr += 1
                    # po[n, k*DK:+DK] = sum_m adjT[m,n].T @ h[m, k*DK:+DK]
                    for nch in range(NC2):
                        for mch in range(NC2):
                            nc.tensor.matmul(
                                po[nch][:, k * DK:(k + 1) * DK],
                                aT[:, mch * N + nch * P:
                                   mch * N + (nch + 1) * P],
                                h_sb[mch][:, k * DK:(k + 1) * DK],
                                start=(mch == 0), stop=(mch == NC2 - 1))

                # --- drain accumulators + store ---
                for nch in range(NC2):
                    ot = p_out.tile([P, OUT_DIM], FP32, tag="o", name=f"o{b}_{nch}")
                    nc.vector.tensor_copy(ot[:], po[nch][:])
                    nc.scalar.dma_start(
                        out=out_ap[b, nch * P:(nch + 1) * P, :], in_=ot[:])

    nc.compile()
    return nc


def _get_nc():
    if "nc" not in _compiled:
        _compiled["nc"] = _build()
    return _compiled["nc"]


def _run(inputs, trace=False, trace_cores=None):
    nc = _get_nc()
    node_feats = np.ascontiguousarray(inputs["node_feats"], dtype=np.float32)
    adj = np.ascontiguousarray(inputs["adj"], dtype=np.float32)
    weight = np.ascontiguousarray(inputs["weight"], dtype=np.float32)
    bias = np.ascontiguousarray(inputs["bias"], dtype=np.float32)
    in_maps = []
    for c in range(N_CORES):
        sl = slice(c * BPC, (c + 1) * BPC)
        in_maps.append({
            "node_feats": node_feats[sl],
            "adj": adj[sl],
            "weight": weight,
            "bias": bias,
        })
    res = run_bass_kernel_spmd(
        nc, in_maps, core_ids=list(range(N_CORES)),
        trace=trace, trace_cores=trace_cores)
    out = np.concatenate([res.results[c]["out"] for c in range(N_CORES)], axis=0)
    return out.reshape(B, N, OUT_DIM), res


def kernel(**inputs) -> np.ndarray:
    return _run(inputs, trace=False)[0]
